# revision 28
# baseline (speedup 1.0000x reference)
"""Trainium2 Bass kernel for the MetaNeuralCV model (dense_mlp), V6:
fp8/bf16 DoubleRow matmuls + a 5-stage latency-hiding software pipeline.

Math (per sample x, score s; MLP 8 -> 256 -> 256 -> 1 -> 8):
    z0 = W0 x + b0;  y0 = tanh(z0)
    z1 = W1 y0 + b1; y1 = tanh(z1)
    z2 = w2.y1 + b2; y2 = tanh(z2)        (w2 = W2[0])
    u  = y2 * w3 + b3                      (w3 = W3[:,0])
    out = c + trace(J) + u.s
The last two layers pass through scalar y2, so the Jacobian is rank-1:
    trace(J) = (1 - y2^2) * q,   q = w2^T D1 W1 D0 (W0 w3)
with D# = diag(1 - y#^2),  a = W0 w3,  Wu = diag(w2) W1 diag(a),
    q = R1 - sum_i y1_i^2 r1_i + sum_i (y1_i^2 - 1) * (Wu y0^2)_i
where r1 = Wu @ ones, R1 = sum(r1); and u.s = y2 * (w3.s) + (b3.s).

Device mapping per core (batch shard BC=8192, data parallel, NT=512):
 - L0 f32r (exact); L1/Wu fp8e4m3 DoubleRow (W1*2^6, Wu*2^9 host-side
   scaling); y1/sq1 in bf16 so the DVE TensorTensor square runs in the
   2x_1p fast mode; sA = -r1.sq1 via plain bf16 matmuls, sB over the
   fp8 wpp = (sq1-1)*u via DoubleRow; z2 = w2.y1 plain bf16 to psum
   row 32 (DoubleRow output must start at partition 0, where q lives).
 - y0^2 on Pool (TensorTensor is the only GPSIMD elementwise op and
   GPSIMD cannot touch PSUM); wpp on DVE (reads PSUM-resident u).
 - 5-stage software pipeline, one tile per stage per slot:
     A: L0 -> y0     B: sq0, L1 -> y1     C: sq1, z2
     D: Wu -> wpp    E: sA+sB -> q row, extract (copy + pair DMAs)
   emitted per slot as A(t), E(t-4), B(t-1), C(t-2), D(t-3) so every
   cross-engine consumer reads tensors produced >= 1 slot earlier
   (real HW loses ~2us/tile to semaphore latency otherwise; same-slot
   deferrals measurably regress).  sq0/sq1 live in a bufs=3 pool since
   their consumers run 2-3 slots after the producer.
 - extraction: per-tile [33,NT] psum->sbuf descale copy on DVE, then
   per-pair rearrange DMAs on the SP queue; tail (y2/q/out) runs in two
   halves so the first half's output DMA overlaps the loop.
Measured ~66-71 us/iter on HW (loop-marginal), rel err 3.9e-3.
"""

import numpy as np
import ml_dtypes

import concourse.bass as bass
import concourse.mybir as mybir
import concourse.tile as tile
from concourse import bacc
from concourse.bass_utils import run_bass_kernel_spmd

B_TOTAL = 65536
D_IN = 8
H = 256
N_CORES = 8
BC = B_TOTAL // N_CORES        # 8192 samples per core
NT_DEFAULT = 512

F32 = mybir.dt.float32
F32R = mybir.dt.float32r
BF16 = mybir.dt.bfloat16
F8 = mybir.dt.float8e4
DR = mybir.MatmulPerfMode.DoubleRow

SW1 = 6     # W1 fp8 scale exponent
SWU = 9     # Wu fp8 scale exponent
SRED = 6    # reduction-row PSUM scale exponent

LAST_RESULT = None


def _build(b2f: float, cf: float, R1f: float, loop_iters: int | None = None,
           NT: int = NT_DEFAULT, bc: int = BC, unroll: int = 1):
    NTILES = bc // NT
    FB = bc // 128
    TPS = NT // FB if NT >= FB else 1   # staging partitions per tile
    nc = bacc.Bacc("TRN2", target_bir_lowering=False, debug=False)
    Tanh = mybir.ActivationFunctionType.Tanh
    Alu = mybir.AluOpType

    xsT = nc.dram_tensor("xsT", [41, bc], F32R, kind="ExternalInput")
    w0t_d = nc.dram_tensor("w0t", [41, H], F32R, kind="ExternalInput")
    w1q_d = nc.dram_tensor("w1q", [128, 2, H], F8, kind="ExternalInput")
    wuq_d = nc.dram_tensor("wuq", [128, 2, H], F8, kind="ExternalInput")
    rlq_d = nc.dram_tensor("rlq", [128, 2, 16], F8, kind="ExternalInput")
    w28_d = nc.dram_tensor("w28", [128, 2], BF16, kind="ExternalInput")
    rA2_d = nc.dram_tensor("rA2", [128, 2, 1], BF16, kind="ExternalInput")
    b1q_d = nc.dram_tensor("b1q", [32, 2, 128], F8, kind="ExternalInput")
    on4_d = nc.dram_tensor("on4", [32, NT], F8, kind="ExternalInput")
    b2c_d = nc.dram_tensor("b2c", [128, 1], F32, kind="ExternalInput")
    p0_d = nc.dram_tensor("p0n", [128, FB], F32, kind="ExternalInput")
    p1_d = nc.dram_tensor("p1n", [128, FB], F32, kind="ExternalInput")
    out_d = nc.dram_tensor("out", [bc], F32, kind="ExternalOutput")

    if bc == 8192:
        CH_BOUNDS = [0, 1024, 3072, 5632, bc]
    else:
        CH_BOUNDS = [0, bc]
    NCHUNK = len(CH_BOUNDS) - 1

    with tile.TileContext(nc) as tc:
        with (
            tc.tile_pool(name="const", bufs=1) as cp,
            tc.tile_pool(name="work", bufs=2) as wp,
            tc.tile_pool(name="work3", bufs=3) as wp3,
            tc.tile_pool(name="stage", bufs=1) as stp,
            tc.tile_pool(name="ps", bufs=2, space="PSUM") as ps,
            tc.tile_pool(name="ps_u", bufs=1, space="PSUM") as pu,
            tc.tile_pool(name="ps_r", bufs=2, space="PSUM") as pr,
        ):
            # sync queue: only what the first tiles need, in order
            w0s = cp.tile([41, H], F32R)
            nc.sync.dma_start(out=w0s[:], in_=w0t_d[:])
            xch = []
            xc0 = cp.tile([41, CH_BOUNDS[1] - CH_BOUNDS[0]], F32R, tag="xch0")
            nc.sync.dma_start(out=xc0[0:9, :], in_=xsT[0:9, 0:CH_BOUNDS[1]])
            nc.scalar.dma_start(out=xc0[32:41, :],
                                in_=xsT[32:41, 0:CH_BOUNDS[1]])
            xch.append(xc0)
            w1s = cp.tile([128, 2, H], F8)
            nc.sync.dma_start(out=w1s[:], in_=w1q_d[:])
            b1s = cp.tile([32, 2, 128], F8)
            nc.sync.dma_start(out=b1s[:], in_=b1q_d[:])
            on4 = cp.tile([32, NT], F8)
            nc.sync.dma_start(out=on4[:], in_=on4_d[:])
            wus = cp.tile([128, 2, H], F8)
            nc.sync.dma_start(out=wus[:], in_=wuq_d[:])
            rls = cp.tile([128, 2, 16], F8)
            nc.sync.dma_start(out=rls[:], in_=rlq_d[:])
            w2s8 = cp.tile([128, 2], BF16)
            nc.sync.dma_start(out=w2s8[:], in_=w28_d[:])
            rA2 = cp.tile([128, 2, 1], BF16)
            nc.sync.dma_start(out=rA2[:], in_=rA2_d[:])
            for ci in range(1, NCHUNK):
                lo, hi = CH_BOUNDS[ci], CH_BOUNDS[ci + 1]
                xc = cp.tile([41, hi - lo], F32R, tag=f"xch{ci}")
                nc.sync.dma_start(out=xc[0:9, :], in_=xsT[0:9, lo:hi])
                nc.scalar.dma_start(out=xc[32:41, :], in_=xsT[32:41, lo:hi])
                xch.append(xc)
            # remaining consts on the Pool SWDGE queue
            b2s = cp.tile([128, 1], F32)
            nc.gpsimd.dma_start(out=b2s[:], in_=b2c_d[:])
            p0s = stp.tile([128, FB], F32)
            nc.gpsimd.dma_start(out=p0s[:], in_=p0_d[:])
            p1s = stp.tile([128, FB], F32)
            nc.gpsimd.dma_start(out=p1s[:], in_=p1_d[:])

            # staging [128, FB]: batch index = p*FB + f
            z2s = stp.tile([128, FB], F32)
            qs = stp.tile([128, FB], F32)

            def body(iv=None):
                st = {}
                widths = [NT] * NTILES
                ncols = [sum(widths[:i]) for i in range(len(widths))]
                NTT = len(widths)

                # 5-stage pipeline: every consumer reads tensors produced at
                # least one slot earlier, so no engine waits on same-slot
                # work from another engine (except y0/y1 chained on ACT).
                def stageA(t):      # L0 -> z0 ; y0 = tanh(z0)
                    col, w = ncols[t], widths[t]
                    ci = next(i for i in range(NCHUNK)
                              if CH_BOUNDS[i + 1] > col)
                    xc = xch[ci]
                    ns = bass.ds(col - CH_BOUNDS[ci], w)
                    z0 = ps.tile([128, 2, NT], F32, tag="zz")
                    nc.tensor.matmul(
                        z0[:, 0, :w], w0s[0:9, 0:128], xc[0:9, ns],
                        start=True, stop=True,
                    )
                    nc.tensor.matmul(
                        z0[:, 1, :w], w0s[32:41, 128:256], xc[32:41, ns],
                        start=True, stop=True,
                    )
                    y0 = wp.tile([128, 2, NT], F8, tag="y0")
                    nc.scalar.activation(y0[:, :, :w], z0[:, :, :w], Tanh)
                    st[("y0", t)] = y0

                def stageB(t):      # sq0 (Pool); L1 -> z1 ; y1
                    y0 = st.pop(("y0", t))
                    w = widths[t]
                    sq0 = wp3.tile([128, 2, NT], F8, tag="sq0")
                    nc.gpsimd.tensor_mul(sq0[:, :, :w], y0[:, :, :w],
                                         y0[:, :, :w])
                    z1 = ps.tile([128, 2, NT], F32, tag="zz")
                    for m in (0, 1):
                        nc.tensor.matmul(
                            z1[:, m, :w], b1s[0:4, m, 0:128], on4[0:4, :w],
                            start=True, stop=False, skip_group_check=True,
                        )
                        nc.tensor.matmul(
                            z1[:, m, :w], w1s[:, :, bass.ts(m, 128)],
                            y0[:, :, :w], start=False, stop=True,
                            perf_mode=DR, skip_group_check=True,
                        )
                    y1 = wp.tile([128, 2, NT], BF16, tag="y1")
                    nc.scalar.activation(y1[:, :, :w], z1[:, :, :w], Tanh,
                                         scale=float(2.0 ** -SW1))
                    st[("sq0", t)] = sq0
                    st[("y1", t)] = y1

                def stageC(t):      # sq1 = y1^2 (DVE); z2 -> red row 32
                    y1 = st.pop(("y1", t))
                    w = widths[t]
                    sq1 = wp3.tile([128, 2, NT], BF16, tag="sq1")
                    nc.vector.tensor_mul(sq1[:, :, :w], y1[:, :, :w],
                                         y1[:, :, :w])
                    red = pr.tile([128, NT], F32, tag="red")
                    st[("red", t)] = red
                    nc.tensor.matmul(red[32:33, :w], w2s8[:, 0:1],
                                     y1[:, 0, :w], start=True, stop=False)
                    nc.tensor.matmul(red[32:33, :w], w2s8[:, 1:2],
                                     y1[:, 1, :w], start=False, stop=True)
                    st[("sq1", t)] = sq1

                def stageD(t):      # u = Wu sq0 ; wpp = (sq1-1)*u
                    sq0 = st.pop(("sq0", t))
                    sq1 = st[("sq1", t)]
                    w = widths[t]
                    u = pu.tile([128, 2, NT], F32, tag="u")
                    for m in (0, 1):
                        nc.tensor.matmul(
                            u[:, m, :w], wus[:, :, bass.ts(m, 128)],
                            sq0[:, :, :w], start=True, stop=True, perf_mode=DR,
                        )
                    wpp = wp.tile([128, 2, NT], F8, tag="wpp")
                    nc.vector.scalar_tensor_tensor(
                        wpp[:, :, :w], sq1[:, :, :w], 1.0, u[:, :, :w],
                        op0=Alu.subtract, op1=Alu.mult,
                    )
                    st[("wpp", t)] = wpp

                def stageE(t):      # sA + sB -> red row 0 ; extract
                    sq1 = st.pop(("sq1", t))
                    wpp = st.pop(("wpp", t))
                    red = st[("red", t)]
                    w = widths[t]
                    nc.tensor.matmul(red[0:1, :w], rA2[:, 0, 0:1],
                                     sq1[:, 0, :w], start=True, stop=False,
                                     skip_group_check=True)
                    nc.tensor.matmul(red[0:1, :w], rA2[:, 1, 0:1],
                                     sq1[:, 1, :w], start=False, stop=False,
                                     skip_group_check=True)
                    nc.tensor.matmul(red[0:1, :w], rls[:, :, 1:2],
                                     wpp[:, :, :w], start=False, stop=True,
                                     perf_mode=DR, skip_group_check=True)
                    extract(t)

                def extract(t):
                    red = st.pop(("red", t))
                    w = widths[t]
                    h = t % 2
                    if h == 0:
                        redsb = wp.tile([33, 2 * NT], F32, tag="redsb")
                        st["redsb"] = redsb
                    redsb = st["redsb"]
                    off = 0 if h == 0 else widths[t - 1]
                    nc.vector.tensor_scalar_mul(
                        redsb[:, off:off + w], red[0:33, :w],
                        float(2.0 ** -SRED))
                    if h == 1 or t == NTT - 1:
                        t0 = t - h
                        ws = off + w
                        p_lo = ncols[t0] // FB
                        p_hi = p_lo + ws // FB
                        nc.sync.dma_start(out=qs[p_lo:p_hi, :],
                                          in_=redsb[0:1, 0:ws])
                        zq = nc.scalar if t >= NTT - 2 else nc.sync
                        zq.dma_start(out=z2s[p_lo:p_hi, :],
                                     in_=redsb[32:33, 0:ws])

                # tail (by halves): q = R1 + sA + sB;
                # out = c + (1-y2^2)q + y2 p0 + p1
                tl = {}

                def tail_half(i):
                    FBH = 64 if NTILES > 1 else 128
                    sl = slice(i * FBH, (i + 1) * FBH)
                    def tile_for(tag):
                        if tag not in tl:
                            tt = stp.tile([128, FB], F32, tag=tag, name=tag)
                            tl[tag] = tt
                        return tl[tag]
                    y2 = tile_for("y2t")
                    nc.scalar.activation(y2[sl, :], z2s[sl, :], Tanh,
                                         bias=b2s[sl, 0:1])
                    q = tile_for("qt")
                    nc.vector.tensor_scalar_add(q[sl, :], qs[sl, :], R1f)
                    t0 = tile_for("t0t")
                    nc.vector.tensor_mul(t0[sl, :], y2[sl, :], y2[sl, :])
                    ndv = tile_for("ndvt")
                    nc.vector.scalar_tensor_tensor(
                        ndv[sl, :], t0[sl, :], 1.0, q[sl, :],
                        op0=Alu.subtract, op1=Alu.mult
                    )  # (y2^2-1)*q = -trace(J)
                    m2 = tile_for("m2t")
                    nc.vector.tensor_mul(m2[sl, :], y2[sl, :], p0s[sl, :])
                    o1 = tile_for("o1t")
                    nc.vector.tensor_sub(o1[sl, :], m2[sl, :], ndv[sl, :])
                    o2 = tile_for("o2t")
                    nc.vector.scalar_tensor_tensor(
                        o2[sl, :], o1[sl, :], cf, p1s[sl, :],
                        op0=Alu.add, op1=Alu.add
                    )
                    nc.sync.dma_start(
                        out=out_d.rearrange("(p f) -> p f", p=128)[sl, :],
                        in_=o2[sl, :]
                    )

                for t in range(NTT + 4):
                    if t < NTT:
                        stageA(t)
                    if 0 <= t - 4:
                        stageE(t - 4)
                    if 0 <= t - 1 < NTT:
                        stageB(t - 1)
                    if 0 <= t - 2 < NTT:
                        stageC(t - 2)
                    if 0 <= t - 3 < NTT:
                        stageD(t - 3)
                    if NTT > 1 and t == 13:
                        tail_half(0)
                tail_half(1 if NTT > 1 else 0)
            if loop_iters is None:
                for _ in range(unroll):
                    body()
            else:
                with tc.For_i(0, loop_iters, 1) as iv:
                    body(iv)

    nc.compile()
    return nc


def build_for_inputs(x_batch, scores_x_batch, W0, b0, W1, b1, W2, b2, W3, b3,
                     c, loop_iters=None, NT=NT_DEFAULT, bc=BC, n_cores=None,
                     unroll=1):
    f = np.float32
    f8 = ml_dtypes.float8_e4m3
    x = np.asarray(x_batch, f)
    s = np.asarray(scores_x_batch, f)
    W0 = np.asarray(W0, f)
    W1 = np.asarray(W1, f)
    W2 = np.asarray(W2, f)
    W3 = np.asarray(W3, f)
    b0 = np.asarray(b0, f)
    b1 = np.asarray(b1, f)
    b3 = np.asarray(b3, f)
    b2f = float(np.asarray(b2, f).reshape(-1)[0])
    cf = float(np.asarray(c, f).reshape(-1)[0])
    if n_cores is None:
        n_cores = N_CORES
    FB = bc // 128

    w2 = W2[0]
    w3 = W3[:, 0]
    a = (W0 @ w3).astype(f)
    Wu = (w2[:, None] * W1 * a[None, :]).astype(f)   # diag(w2) W1 diag(a)
    r1 = Wu.sum(axis=1).astype(f)
    R1f = float(r1.sum())

    def ktiles(M):  # [256, 256] -> [128, 2, 256] k-tile layout
        return np.ascontiguousarray(np.stack([M[0:128], M[128:256]], axis=1))

    def cols(v):  # [256] -> [128, 2]
        return np.ascontiguousarray(np.stack([v[0:128], v[128:256]], axis=1))

    b1q = np.zeros([32, 2, 128], f)
    for h in (0, 1):
        b1q[0:4, h, :] = b1[h * 128:(h + 1) * 128] * (2.0 ** SW1 / 4.0)
    on4h = np.zeros([32, NT], f)
    on4h[0:4] = 1.0

    w0t = np.zeros([41, H], f)
    w0t[0:8] = W0.T
    w0t[8] = b0
    w0t[32:40] = W0.T
    w0t[40] = b0

    rlq = np.zeros([128, 2, 16], f)
    rlq[:, :, 0] = cols(-r1 * 2.0 ** SRED)   # unused (sA now plain bf16)
    rlq[:, :, 1] = 2.0 ** (SRED - SWU)

    common = {
        "w0t": w0t,
        "w1q": ktiles(W1.T * 2.0 ** SW1).astype(f8),
        "wuq": ktiles(Wu.T * 2.0 ** SWU).astype(f8),
        "rlq": rlq.astype(f8),
        "w28": cols(w2 * 2.0 ** SRED).astype(ml_dtypes.bfloat16),
        "rA2": cols(-r1 * 2.0 ** SRED).astype(ml_dtypes.bfloat16)[:, :, None],
        "b1q": b1q.astype(f8),
        "on4": on4h.astype(f8),
        "b2c": np.full([128, 1], b2f, f),
    }

    nc = _build(b2f, cf, R1f, loop_iters=loop_iters, NT=NT, bc=bc,
                unroll=unroll)

    p0 = (s @ w3).astype(f)
    p1 = (s @ b3).astype(f)

    in_maps = []
    for i in range(n_cores):
        m = dict(common)
        sl = slice(i * bc, (i + 1) * bc)
        xT = np.ascontiguousarray(x[sl].T)
        xs = np.zeros([41, bc], f)
        xs[0:8] = xT
        xs[8] = 1.0
        xs[32:40] = xT
        xs[40] = 1.0
        m["xsT"] = xs
        m["p0n"] = np.ascontiguousarray(p0[sl].reshape(128, FB))
        m["p1n"] = np.ascontiguousarray(p1[sl].reshape(128, FB))
        in_maps.append(m)

    return nc, in_maps


def kernel(x_batch, scores_x_batch, W0, b0, W1, b1, W2, b2, W3, b3, c):
    global LAST_RESULT
    nc, in_maps = build_for_inputs(x_batch, scores_x_batch, W0, b0, W1, b1,
                                   W2, b2, W3, b3, c)
    res = run_bass_kernel_spmd(nc, in_maps, core_ids=list(range(N_CORES)))
    LAST_RESULT = res
    return np.concatenate([r["out"] for r in res.results]).astype(np.float32)



# revision 29
# speedup vs baseline: 1.6157x; 1.6157x over previous
"""Trainium2 Bass kernel for the MetaNeuralCV model (dense_mlp), V6:
fp8/bf16 DoubleRow matmuls + a 5-stage latency-hiding software pipeline.

Math (per sample x, score s; MLP 8 -> 256 -> 256 -> 1 -> 8):
    z0 = W0 x + b0;  y0 = tanh(z0)
    z1 = W1 y0 + b1; y1 = tanh(z1)
    z2 = w2.y1 + b2; y2 = tanh(z2)        (w2 = W2[0])
    u  = y2 * w3 + b3                      (w3 = W3[:,0])
    out = c + trace(J) + u.s
The last two layers pass through scalar y2, so the Jacobian is rank-1:
    trace(J) = (1 - y2^2) * q,   q = w2^T D1 W1 D0 (W0 w3)
with D# = diag(1 - y#^2),  a = W0 w3,  Wu = diag(w2) W1 diag(a),
    q = R1 - sum_i y1_i^2 r1_i + sum_i (y1_i^2 - 1) * (Wu y0^2)_i
where r1 = Wu @ ones, R1 = sum(r1); and u.s = y2 * (w3.s) + (b3.s).

Device mapping per core (batch shard BC=8192, data parallel, NT=512):
 - L0 f32r (exact); L1/Wu fp8e4m3 DoubleRow (W1*2^6, Wu*2^9 host-side
   scaling); y1/sq1 in bf16 so the DVE TensorTensor square runs in the
   2x_1p fast mode; sA = -r1.sq1 via plain bf16 matmuls, sB over the
   fp8 wpp = (sq1-1)*u via DoubleRow; z2 = w2.y1 plain bf16 to psum
   row 32 (DoubleRow output must start at partition 0, where q lives).
 - y0^2 on Pool (TensorTensor is the only GPSIMD elementwise op and
   GPSIMD cannot touch PSUM); wpp on DVE (reads PSUM-resident u).
 - 5-stage software pipeline, one tile per stage per slot:
     A: L0 -> y0     B: sq0, L1 -> y1     C: sq1, z2
     D: Wu -> wpp    E: sA+sB -> q row, extract (copy + pair DMAs)
   emitted per slot as A(t), E(t-4), B(t-1), C(t-2), D(t-3) so every
   cross-engine consumer reads tensors produced >= 1 slot earlier
   (real HW loses ~2us/tile to semaphore latency otherwise; same-slot
   deferrals measurably regress).  sq0/sq1 live in a bufs=3 pool since
   their consumers run 2-3 slots after the producer.
 - extraction: per-tile [33,NT] psum->sbuf descale copy on DVE, then
   per-pair rearrange DMAs on the SP queue; tail (y2/q/out) runs in two
   halves so the first half's output DMA overlaps the loop.
Measured ~66-71 us/iter on HW (loop-marginal), rel err 3.9e-3.
"""

import numpy as np
import ml_dtypes

import concourse.bass as bass
import concourse.mybir as mybir
import concourse.tile as tile
from concourse import bacc
from concourse.bass_utils import run_bass_kernel_spmd

B_TOTAL = 65536
D_IN = 8
H = 256
N_CORES = 8
BC = B_TOTAL // N_CORES        # 8192 samples per core
NT_DEFAULT = 512

F32 = mybir.dt.float32
F32R = mybir.dt.float32r
BF16 = mybir.dt.bfloat16
F8 = mybir.dt.float8e4
DR = mybir.MatmulPerfMode.DoubleRow

SW1 = 6     # W1 fp8 scale exponent
SWU = 9     # Wu fp8 scale exponent
SRED = 6    # reduction-row PSUM scale exponent

LAST_RESULT = None


def _build(b2f: float, cf: float, R1f: float, loop_iters: int | None = None,
           NT: int = NT_DEFAULT, bc: int = BC, unroll: int = 1):
    NTILES = bc // NT
    FB = bc // 128
    TPS = NT // FB if NT >= FB else 1   # staging partitions per tile
    nc = bacc.Bacc("TRN2", target_bir_lowering=False, debug=False)
    Tanh = mybir.ActivationFunctionType.Tanh
    Alu = mybir.AluOpType

    xsT = nc.dram_tensor("xsT", [41, bc], F32R, kind="ExternalInput")
    w0t_d = nc.dram_tensor("w0t", [41, H], F32R, kind="ExternalInput")
    w1q_d = nc.dram_tensor("w1q", [128, 2, H], F8, kind="ExternalInput")
    wuq_d = nc.dram_tensor("wuq", [128, 2, H], F8, kind="ExternalInput")
    rlq_d = nc.dram_tensor("rlq", [128, 2, 16], F8, kind="ExternalInput")
    w28_d = nc.dram_tensor("w28", [128, 2], BF16, kind="ExternalInput")
    rA2_d = nc.dram_tensor("rA2", [128, 2, 1], BF16, kind="ExternalInput")
    b1c_d = nc.dram_tensor("b1c", [128, 2], F32, kind="ExternalInput")
    b2c_d = nc.dram_tensor("b2c", [128, 1], F32, kind="ExternalInput")
    p0_d = nc.dram_tensor("p0n", [128, FB], F32, kind="ExternalInput")
    p1_d = nc.dram_tensor("p1n", [128, FB], F32, kind="ExternalInput")
    out_d = nc.dram_tensor("out", [bc], F32, kind="ExternalOutput")

    if bc == 8192:
        CH_BOUNDS = [0, 1024, 3072, 5632, bc]
    else:
        CH_BOUNDS = [0, bc]
    NCHUNK = len(CH_BOUNDS) - 1

    with tile.TileContext(nc) as tc:
        with (
            tc.tile_pool(name="const", bufs=1) as cp,
            tc.tile_pool(name="work", bufs=2) as wp,
            tc.tile_pool(name="work3", bufs=3) as wp3,
            tc.tile_pool(name="stage", bufs=1) as stp,
            tc.tile_pool(name="ps", bufs=2, space="PSUM") as ps,
            tc.tile_pool(name="ps_u", bufs=1, space="PSUM") as pu,
            tc.tile_pool(name="ps_r", bufs=2, space="PSUM") as pr,
        ):
            # sync queue: only what the first tiles need, in order
            w0s = cp.tile([41, H], F32R)
            nc.sync.dma_start(out=w0s[:], in_=w0t_d[:])
            xch = []
            xc0 = cp.tile([41, CH_BOUNDS[1] - CH_BOUNDS[0]], F32R, tag="xch0")
            nc.sync.dma_start(out=xc0[0:9, :], in_=xsT[0:9, 0:CH_BOUNDS[1]])
            nc.scalar.dma_start(out=xc0[32:41, :],
                                in_=xsT[32:41, 0:CH_BOUNDS[1]])
            xch.append(xc0)
            w1s = cp.tile([128, 2, H], F8)
            nc.sync.dma_start(out=w1s[:], in_=w1q_d[:])
            b1s = cp.tile([128, 2], F32)
            nc.sync.dma_start(out=b1s[:], in_=b1c_d[:])
            wus = cp.tile([128, 2, H], F8)
            nc.sync.dma_start(out=wus[:], in_=wuq_d[:])
            rls = cp.tile([128, 2, 16], F8)
            nc.sync.dma_start(out=rls[:], in_=rlq_d[:])
            w2s8 = cp.tile([128, 2], BF16)
            nc.sync.dma_start(out=w2s8[:], in_=w28_d[:])
            rA2 = cp.tile([128, 2, 1], BF16)
            nc.sync.dma_start(out=rA2[:], in_=rA2_d[:])
            for ci in range(1, NCHUNK):
                lo, hi = CH_BOUNDS[ci], CH_BOUNDS[ci + 1]
                xc = cp.tile([41, hi - lo], F32R, tag=f"xch{ci}")
                nc.sync.dma_start(out=xc[0:9, :], in_=xsT[0:9, lo:hi])
                nc.scalar.dma_start(out=xc[32:41, :], in_=xsT[32:41, lo:hi])
                xch.append(xc)
            # remaining consts on the Pool SWDGE queue
            b2s = cp.tile([128, 1], F32)
            nc.gpsimd.dma_start(out=b2s[:], in_=b2c_d[:])
            p0s = stp.tile([128, FB], F32)
            nc.gpsimd.dma_start(out=p0s[:], in_=p0_d[:])
            p1s = stp.tile([128, FB], F32)
            nc.gpsimd.dma_start(out=p1s[:], in_=p1_d[:])

            # staging [128, FB]: batch index = p*FB + f
            z2s = stp.tile([128, FB], F32)
            qs = stp.tile([128, FB], F32)

            def body(iv=None):
                st = {}
                widths = [NT] * NTILES
                ncols = [sum(widths[:i]) for i in range(len(widths))]
                NTT = len(widths)

                # 5-stage pipeline: every consumer reads tensors produced at
                # least one slot earlier, so no engine waits on same-slot
                # work from another engine (except y0/y1 chained on ACT).
                def stageA(t):      # L0 -> z0 ; y0 = tanh(z0)
                    col, w = ncols[t], widths[t]
                    ci = next(i for i in range(NCHUNK)
                              if CH_BOUNDS[i + 1] > col)
                    xc = xch[ci]
                    ns = bass.ds(col - CH_BOUNDS[ci], w)
                    z0 = ps.tile([128, 2, NT], F32, tag="zz")
                    nc.tensor.matmul(
                        z0[:, 0, :w], w0s[0:9, 0:128], xc[0:9, ns],
                        start=True, stop=True,
                    )
                    nc.tensor.matmul(
                        z0[:, 1, :w], w0s[32:41, 128:256], xc[32:41, ns],
                        start=True, stop=True,
                    )
                    y0 = wp.tile([128, 2, NT], F8, tag="y0")
                    nc.scalar.activation(y0[:, :, :w], z0[:, :, :w], Tanh)
                    st[("y0", t)] = y0

                def stageB(t):      # sq0 (Pool); L1 -> z1 ; y1
                    y0 = st.pop(("y0", t))
                    w = widths[t]
                    sq0 = wp3.tile([128, 2, NT], F8, tag="sq0")
                    nc.gpsimd.tensor_mul(sq0[:, :, :w], y0[:, :, :w],
                                         y0[:, :, :w])
                    z1 = ps.tile([128, 2, NT], F32, tag="zz")
                    for m in (0, 1):
                        nc.tensor.matmul(
                            z1[:, m, :w], w1s[:, :, bass.ts(m, 128)],
                            y0[:, :, :w], start=True, stop=True, perf_mode=DR,
                        )
                    y1 = wp.tile([128, 2, NT], BF16, tag="y1")
                    for h in (0, 1):
                        nc.scalar.activation(
                            y1[:, h, :w], z1[:, h, :w], Tanh,
                            bias=b1s[:, h:h + 1], scale=float(2.0 ** -SW1),
                        )
                    st[("sq0", t)] = sq0
                    st[("y1", t)] = y1

                def stageC(t):      # sq1 = y1^2 (DVE); z2 -> red row 32
                    y1 = st.pop(("y1", t))
                    w = widths[t]
                    sq1 = wp3.tile([128, 2, NT], BF16, tag="sq1")
                    nc.vector.tensor_mul(sq1[:, :, :w], y1[:, :, :w],
                                         y1[:, :, :w])
                    red = pr.tile([128, NT], F32, tag="red")
                    st[("red", t)] = red
                    nc.tensor.matmul(red[32:33, :w], w2s8[:, 0:1],
                                     y1[:, 0, :w], start=True, stop=False)
                    nc.tensor.matmul(red[32:33, :w], w2s8[:, 1:2],
                                     y1[:, 1, :w], start=False, stop=True)
                    st[("sq1", t)] = sq1

                def stageD(t):      # u = Wu sq0 ; wpp = (sq1-1)*u
                    sq0 = st.pop(("sq0", t))
                    sq1 = st[("sq1", t)]
                    w = widths[t]
                    u = pu.tile([128, 2, NT], F32, tag="u")
                    for m in (0, 1):
                        nc.tensor.matmul(
                            u[:, m, :w], wus[:, :, bass.ts(m, 128)],
                            sq0[:, :, :w], start=True, stop=True, perf_mode=DR,
                        )
                    wpp = wp.tile([128, 2, NT], F8, tag="wpp")
                    nc.vector.scalar_tensor_tensor(
                        wpp[:, :, :w], sq1[:, :, :w], 1.0, u[:, :, :w],
                        op0=Alu.subtract, op1=Alu.mult,
                    )
                    st[("wpp", t)] = wpp

                def stageE(t):      # sA + sB -> red row 0 ; extract
                    sq1 = st.pop(("sq1", t))
                    wpp = st.pop(("wpp", t))
                    red = st[("red", t)]
                    w = widths[t]
                    nc.tensor.matmul(red[0:1, :w], rA2[:, 0, 0:1],
                                     sq1[:, 0, :w], start=True, stop=False,
                                     skip_group_check=True)
                    nc.tensor.matmul(red[0:1, :w], rA2[:, 1, 0:1],
                                     sq1[:, 1, :w], start=False, stop=False,
                                     skip_group_check=True)
                    nc.tensor.matmul(red[0:1, :w], rls[:, :, 1:2],
                                     wpp[:, :, :w], start=False, stop=True,
                                     perf_mode=DR, skip_group_check=True)
                    extract(t)

                def extract(t):
                    red = st.pop(("red", t))
                    w = widths[t]
                    h = t % 2
                    if h == 0:
                        redsb = wp.tile([33, 2 * NT], F32, tag="redsb")
                        st["redsb"] = redsb
                    redsb = st["redsb"]
                    off = 0 if h == 0 else widths[t - 1]
                    nc.vector.tensor_scalar_mul(
                        redsb[:, off:off + w], red[0:33, :w],
                        float(2.0 ** -SRED))
                    if h == 1 or t == NTT - 1:
                        t0 = t - h
                        ws = off + w
                        p_lo = ncols[t0] // FB
                        p_hi = p_lo + ws // FB
                        nc.sync.dma_start(out=qs[p_lo:p_hi, :],
                                          in_=redsb[0:1, 0:ws])
                        zq = nc.scalar if t >= NTT - 2 else nc.sync
                        zq.dma_start(out=z2s[p_lo:p_hi, :],
                                     in_=redsb[32:33, 0:ws])

                # tail (by halves): q = R1 + sA + sB;
                # out = c + (1-y2^2)q + y2 p0 + p1
                tl = {}

                def tail_half(i):
                    FBH = 64 if NTILES > 1 else 128
                    sl = slice(i * FBH, (i + 1) * FBH)
                    def tile_for(tag):
                        if tag not in tl:
                            tt = stp.tile([128, FB], F32, tag=tag, name=tag)
                            tl[tag] = tt
                        return tl[tag]
                    y2 = tile_for("y2t")
                    nc.scalar.activation(y2[sl, :], z2s[sl, :], Tanh,
                                         bias=b2s[sl, 0:1])
                    q = tile_for("qt")
                    nc.vector.tensor_scalar_add(q[sl, :], qs[sl, :], R1f)
                    t0 = tile_for("t0t")
                    nc.vector.tensor_mul(t0[sl, :], y2[sl, :], y2[sl, :])
                    ndv = tile_for("ndvt")
                    nc.vector.scalar_tensor_tensor(
                        ndv[sl, :], t0[sl, :], 1.0, q[sl, :],
                        op0=Alu.subtract, op1=Alu.mult
                    )  # (y2^2-1)*q = -trace(J)
                    m2 = tile_for("m2t")
                    nc.vector.tensor_mul(m2[sl, :], y2[sl, :], p0s[sl, :])
                    o1 = tile_for("o1t")
                    nc.vector.tensor_sub(o1[sl, :], m2[sl, :], ndv[sl, :])
                    o2 = tile_for("o2t")
                    nc.vector.scalar_tensor_tensor(
                        o2[sl, :], o1[sl, :], cf, p1s[sl, :],
                        op0=Alu.add, op1=Alu.add
                    )
                    nc.sync.dma_start(
                        out=out_d.rearrange("(p f) -> p f", p=128)[sl, :],
                        in_=o2[sl, :]
                    )

                for t in range(NTT + 4):
                    if t < NTT:
                        stageA(t)
                    if 0 <= t - 4:
                        stageE(t - 4)
                    if 0 <= t - 1 < NTT:
                        stageB(t - 1)
                    if 0 <= t - 2 < NTT:
                        stageC(t - 2)
                    if 0 <= t - 3 < NTT:
                        stageD(t - 3)
                    if NTT > 1 and t == 13:
                        tail_half(0)
                tail_half(1 if NTT > 1 else 0)
            if loop_iters is None:
                for _ in range(unroll):
                    body()
            else:
                with tc.For_i(0, loop_iters, 1) as iv:
                    body(iv)

    nc.compile()
    return nc


def build_for_inputs(x_batch, scores_x_batch, W0, b0, W1, b1, W2, b2, W3, b3,
                     c, loop_iters=None, NT=NT_DEFAULT, bc=BC, n_cores=None,
                     unroll=1):
    f = np.float32
    f8 = ml_dtypes.float8_e4m3
    x = np.asarray(x_batch, f)
    s = np.asarray(scores_x_batch, f)
    W0 = np.asarray(W0, f)
    W1 = np.asarray(W1, f)
    W2 = np.asarray(W2, f)
    W3 = np.asarray(W3, f)
    b0 = np.asarray(b0, f)
    b1 = np.asarray(b1, f)
    b3 = np.asarray(b3, f)
    b2f = float(np.asarray(b2, f).reshape(-1)[0])
    cf = float(np.asarray(c, f).reshape(-1)[0])
    if n_cores is None:
        n_cores = N_CORES
    FB = bc // 128

    w2 = W2[0]
    w3 = W3[:, 0]
    a = (W0 @ w3).astype(f)
    Wu = (w2[:, None] * W1 * a[None, :]).astype(f)   # diag(w2) W1 diag(a)
    r1 = Wu.sum(axis=1).astype(f)
    R1f = float(r1.sum())

    def ktiles(M):  # [256, 256] -> [128, 2, 256] k-tile layout
        return np.ascontiguousarray(np.stack([M[0:128], M[128:256]], axis=1))

    def cols(v):  # [256] -> [128, 2]
        return np.ascontiguousarray(np.stack([v[0:128], v[128:256]], axis=1))

    w0t = np.zeros([41, H], f)
    w0t[0:8] = W0.T
    w0t[8] = b0
    w0t[32:40] = W0.T
    w0t[40] = b0

    rlq = np.zeros([128, 2, 16], f)
    rlq[:, :, 0] = cols(-r1 * 2.0 ** SRED)   # unused (sA now plain bf16)
    rlq[:, :, 1] = 2.0 ** (SRED - SWU)

    common = {
        "w0t": w0t,
        "w1q": ktiles(W1.T * 2.0 ** SW1).astype(f8),
        "wuq": ktiles(Wu.T * 2.0 ** SWU).astype(f8),
        "rlq": rlq.astype(f8),
        "w28": cols(w2 * 2.0 ** SRED).astype(ml_dtypes.bfloat16),
        "rA2": cols(-r1 * 2.0 ** SRED).astype(ml_dtypes.bfloat16)[:, :, None],
        "b1c": cols(b1),
        "b2c": np.full([128, 1], b2f, f),
    }

    nc = _build(b2f, cf, R1f, loop_iters=loop_iters, NT=NT, bc=bc,
                unroll=unroll)

    p0 = (s @ w3).astype(f)
    p1 = (s @ b3).astype(f)

    in_maps = []
    for i in range(n_cores):
        m = dict(common)
        sl = slice(i * bc, (i + 1) * bc)
        xT = np.ascontiguousarray(x[sl].T)
        xs = np.zeros([41, bc], f)
        xs[0:8] = xT
        xs[8] = 1.0
        xs[32:40] = xT
        xs[40] = 1.0
        m["xsT"] = xs
        m["p0n"] = np.ascontiguousarray(p0[sl].reshape(128, FB))
        m["p1n"] = np.ascontiguousarray(p1[sl].reshape(128, FB))
        in_maps.append(m)

    return nc, in_maps


def kernel(x_batch, scores_x_batch, W0, b0, W1, b1, W2, b2, W3, b3, c):
    global LAST_RESULT
    nc, in_maps = build_for_inputs(x_batch, scores_x_batch, W0, b0, W1, b1,
                                   W2, b2, W3, b3, c)
    res = run_bass_kernel_spmd(nc, in_maps, core_ids=list(range(N_CORES)))
    LAST_RESULT = res
    return np.concatenate([r["out"] for r in res.results]).astype(np.float32)

